# revision 21
# baseline (speedup 1.0000x reference)
"""Multi-head causal attention (B=4, N=2048, C=1024, H=16) on 8 trn2 NeuronCores.

Sharding: core c -> batch b = c//2, head-group g = c%2 (8 heads each).
Each core computes qkv projection for its heads, causal attention, and a
partial output projection over its 512 attention channels; a pair-wise
ReduceScatter(add) completes the projection, each core emitting its half of
the tokens for its batch.  Host assembles the 8 [1024, 1024] results.

v2: fused per-512-token-group pipeline (DMA -> transpose -> QKV slice ->
attention chunk -> proj -> RS) keeps the PE continuously busy (avoids HAM
clock-gate throttling) and overlaps ScalarE softmax-exp with PE GEMMs
throughout.  bf16 data plane everywhere outside PSUM.  K-bias dropped
(softmax shift invariance), V-bias folded into b_o on host (softmax rows
sum to 1), output bias added on DVE from a broadcast tile.  Softmax
normalization uses gpsimd partition_broadcast (no DRAM round-trip).
"""

import os
import sys

for _p in ("/opt/trn_rl_repo",):
    if _p not in sys.path:
        sys.path.insert(0, _p)

import numpy as np

B = 4
N = 2048
C = 1024
H = 16
DK = 64
N_CORES = 8
HL = 8  # local heads per core
CL = HL * DK  # 512 local channels
PAIRS = HL // 2  # local head pairs
NT = N // 128  # 16 token tiles of 128
NQ = N // 512  # 4 query chunks of 512 (= pipeline groups)
KC = C // 128  # 8 embed contraction chunks

_nc_cache = None


def _build():
    import concourse.bass as bass
    import concourse.mybir as mybir
    import concourse.tile as tile
    from concourse import bacc
    from contextlib import ExitStack

    f32 = mybir.dt.float32
    f32r = mybir.dt.float32r
    bf16 = mybir.dt.bfloat16

    def _r(ap):
        return ap.bitcast(f32r)

    nc = bacc.Bacc("TRN2", target_bir_lowering=False, num_devices=N_CORES)

    xt_d = nc.dram_tensor("xt", [KC, 128, N], bf16, kind="ExternalInput")
    w_q = nc.dram_tensor("w_q", [PAIRS, 128, KC, 128], bf16, kind="ExternalInput")
    w_k = nc.dram_tensor("w_k", [PAIRS, 128, KC, 128], bf16, kind="ExternalInput")
    w_v = nc.dram_tensor("w_v", [KC, 128, CL], bf16, kind="ExternalInput")
    w_o = nc.dram_tensor("w_o", [PAIRS, 128, C], bf16, kind="ExternalInput")
    b_q = nc.dram_tensor("b_q", [128, PAIRS], f32, kind="ExternalInput")
    b_o2 = nc.dram_tensor("b_o2", [1, C], f32, kind="ExternalInput")
    tri_d = nc.dram_tensor("tri", [128, 128], bf16, kind="ExternalInput")
    out = nc.dram_tensor("out", [768 + 512, C], bf16, kind="ExternalOutput")

    EXP = mybir.ActivationFunctionType.Exp

    with tile.TileContext(nc, pool_alloc_mode="queue") as tc, ExitStack() as st:
        # ---------- permanent pools ----------
        const = st.enter_context(tc.tile_pool(name="const", bufs=1))
        tri_sb = const.tile([128, 128], bf16)
        nc.sync.dma_start(out=tri_sb, in_=tri_d[:, :])
        bq_sb = const.tile([128, PAIRS], f32)
        nc.sync.dma_start(out=bq_sb, in_=b_q[:, :])
        bo_bc = const.tile([128, C], f32)
        nc.sync.dma_start(out=bo_bc, in_=b_o2[0:1, :].partition_broadcast(128))

        # resident weights (bf16)
        w_pool = st.enter_context(tc.tile_pool(name="w", bufs=1))
        wq_sb = [w_pool.tile([128, KC, 128], bf16, tag=f"wq{p}", name=f"wq{p}") for p in range(PAIRS)]
        wk_sb = [w_pool.tile([128, KC, 128], bf16, tag=f"wk{p}", name=f"wk{p}") for p in range(PAIRS)]
        wv_sb = [w_pool.tile([128, CL], bf16, tag=f"wv{k}", name=f"wv{k}") for k in range(KC)]
        wo_sb = [w_pool.tile([128, C], bf16, tag=f"wo{c}", name=f"wo{c}") for c in range(PAIRS)]
        for k in range(KC):
            nc.gpsimd.dma_start(out=wv_sb[k], in_=w_v[k])
        for p in range(PAIRS):
            nc.gpsimd.dma_start(out=wq_sb[p], in_=w_q[p])
            nc.gpsimd.dma_start(out=wk_sb[p], in_=w_k[p])
        for p in range(PAIRS):
            nc.gpsimd.dma_start(out=wo_sb[p], in_=w_o[p])

        # persistent activations
        act = st.enter_context(tc.tile_pool(name="act", bufs=1))
        kT = [act.tile([128, N], bf16, tag=f"kT{p}", name=f"kT{p}") for p in range(PAIRS)]
        vt = act.tile([128, NT, HL, DK + 1], bf16, tag="vt", name="vt")
        nc.vector.memset(vt[:, :, :, DK : DK + 1], 1.0)

        xt_pool = st.enter_context(tc.tile_pool(name="xt", bufs=1))
        pt_pool = st.enter_context(tc.tile_pool(name="pt", bufs=4))
        aoT_pool = st.enter_context(tc.tile_pool(name="aoT", bufs=2))
        nrm_pool = st.enter_context(tc.tile_pool(name="nrm", bufs=3))
        ob_pool = st.enter_context(tc.tile_pool(name="ob", bufs=3))

        ps = st.enter_context(tc.tile_pool(name="ps", bufs=1, space="PSUM"))
        dram = st.enter_context(tc.tile_pool(name="dram", bufs=1, space="DRAM"))
        rs_in = dram.tile([N, C], bf16, name="rs_in")
        rs_out = dram.tile([N // 2, C], bf16, name="rs_out")


        tri2 = bass.AP(
            tensor=tri_sb.tensor,
            offset=tri_sb.offset,
            ap=[list(tri_sb.ap[0]), [0, 2], list(tri_sb.ap[1])],
        )

        # psum rotation: small accumulation groups cycle over the same tags
        # attention uses, so all 8 banks serve every phase.
        _grp = [0]

        def psum_grp():
            tag, bufs = (("s", 2), ("ao", 4), ("ao", 4))[_grp[0] % 3]
            _grp[0] += 1
            return ps.tile([128, 512], f32, tag=tag, bufs=bufs, name="pg")

        # x^T slices land per group (double-buffered), group-major so group 0
        # can start as soon as its 512-token slice is in.
        xg = {}

        def load_xt(g):
            tiles = [
                xt_pool.tile([128, 512], bf16, tag=f"xt{k}", bufs=2, name=f"xt{g}_{k}")
                for k in range(KC)
            ]
            for kc in range(KC):
                nc.sync.dma_start(
                    out=tiles[kc][:, :], in_=xt_d[kc][:, g * 512 : (g + 1) * 512]
                )
            xg[g] = tiles

        qT = [
            act.tile([128, 512], bf16, tag=f"qT{p}", name=f"qT{p}")
            for p in range(PAIRS)
        ]
        load_xt(0)
        load_xt(1)

        for g in range(NQ):
            g0 = g * 512
            _grp[0] = 0  # first QKV psum claims land on "s" (freed by exps),
            # not "ao" (held until the previous group's last normalize)
            if g + 2 < NQ:
                load_xt(g + 2)
            xs = xg[g]
            # ---- V slice: natural [tok, chan]; ones column pre-set ----
            for i in range(4):
                mt = 4 * g + i
                pv = psum_grp()
                for kc in range(KC):
                    nc.tensor.matmul(
                        pv[:, :],
                        xs[kc][:, i * 128 : (i + 1) * 128],
                        wv_sb[kc][:, :],
                        start=(kc == 0), stop=(kc == KC - 1),
                    )
                nc.vector.tensor_copy(
                    vt[:, mt, :, 0:DK], pv.rearrange("p (h d) -> p h d", h=HL)
                )

            # ---- Q^T (bias via per-partition add) and K^T (bias dropped) ----
            for p in range(PAIRS):
                pq = psum_grp()
                for kc in range(KC):
                    nc.tensor.matmul(
                        pq[:, :], wq_sb[p][:, kc, :], xs[kc][:, :],
                        start=(kc == 0), stop=(kc == KC - 1),
                    )
                nc.vector.tensor_scalar(
                    out=qT[p][:, :], in0=pq[:, :],
                    scalar1=bq_sb[:, p : p + 1], scalar2=None,
                    op0=mybir.AluOpType.add,
                )

                pk = psum_grp()
                for kc in range(KC):
                    nc.tensor.matmul(
                        pk[:, :], wk_sb[p][:, kc, :], xs[kc][:, :],
                        start=(kc == 0), stop=(kc == KC - 1),
                    )
                nc.vector.tensor_copy(kT[p][:, g0 : g0 + 512], pk[:, :])

            # ---- attention for q-chunk g ----
            aoT = [
                aoT_pool.tile([128, 512], bf16, tag=f"aoq{p}", name=f"aoT{p}_{g}")
                for p in range(PAIRS)
            ]
            for p in range(PAIRS):
                ao = [
                    ps.tile([65, 512], f32, tag="ao", bufs=4, name=f"aops{h}")
                    for h in range(2)
                ]
                n_kt = 4 * g + 4
                for kt in range(n_kt):
                    off = 128 * (kt - 4 * g) if kt >= 4 * g else 0
                    s_t = ps.tile([128, 1024], f32, tag="s", bufs=2, name="st")
                    for h in range(2):
                        rows = slice(64 * h, 64 * h + 64)
                        nc.tensor.matmul(
                            s_t[:, 512 * h + off : 512 * h + 512],
                            kT[p][rows, kt * 128 : (kt + 1) * 128],
                            qT[p][rows, off:512],
                            start=True, stop=True, tile_position=(64 * h, 0),
                        )
                    pt = pt_pool.tile([128, 1024], bf16, name="pt")
                    if off:
                        s3 = bass.AP(
                            tensor=s_t.tensor,
                            offset=s_t.offset + off,
                            ap=[list(s_t.ap[0]), [512, 2], [1, 512 - off]],
                        )
                        p3 = bass.AP(
                            tensor=pt.tensor,
                            offset=pt.offset + off,
                            ap=[list(pt.ap[0]), [512, 2], [1, 512 - off]],
                        )
                        nc.scalar.activation(p3, s3, EXP, scale=0.125)
                    else:
                        nc.scalar.activation(
                            pt[:, 0:1024], s_t[:, 0:1024], EXP, scale=0.125
                        )
                    if kt >= 4 * g:  # triangular boundary blocks, both heads
                        blk = bass.AP(
                            tensor=pt.tensor,
                            offset=pt.offset + off,
                            ap=[list(pt.ap[0]), [512, 2], [1, 128]],
                        )
                        nc.vector.tensor_tensor(
                            blk, blk, tri2, mybir.AluOpType.mult
                        )
                    for h in range(2):
                        nc.tensor.matmul(
                            ao[h][0:65, off:512],
                            vt[:, kt, 2 * p + h, :],
                            pt[:, 512 * h + off : 512 * h + 512],
                            start=(kt == 0), stop=(kt == n_kt - 1),
                            skip_group_check=True,
                        )
                # softmax normalize: aoT = ao[0:64] * (1/rowsum), rowsum = ao[64]
                for h in range(2):
                    rs_row = nrm_pool.tile([1, 512], f32, tag="rsr", bufs=3, name="rsr")
                    nc.vector.tensor_copy(rs_row[0:1, :], ao[h][64:65, :])
                    rcp = nrm_pool.tile([1, 512], f32, tag="rcp", bufs=3, name="rcp")
                    nc.vector.reciprocal_approx_fast(rcp[:, :], rs_row[0:1, :])
                    rcpb = nrm_pool.tile([64, 512], f32, tag="rcpb", bufs=3, name="rcpb")
                    nc.gpsimd.partition_broadcast(rcpb[:, :], rcp[0:1, :], channels=64)
                    nc.vector.tensor_tensor(
                        aoT[p][64 * h : 64 * h + 64, :],
                        ao[h][0:64, :],
                        rcpb[:, :],
                        mybir.AluOpType.mult,
                    )

            # ---- output projection + bias, then pair ReduceScatter ----
            for i in range(4):
                mt = 4 * g + i
                for nn in range(2):
                    pj = ps.tile([128, 512], f32, tag="s", bufs=2, name=f"pj{mt}")
                    for cc in range(PAIRS):
                        nc.tensor.matmul(
                            pj[:, :],
                            aoT[cc][:, i * 128 : (i + 1) * 128],
                            wo_sb[cc][:, nn * 512 : (nn + 1) * 512],
                            start=(cc == 0), stop=(cc == PAIRS - 1),
                        )
                    ob = ob_pool.tile([128, 512], bf16, name="ob")
                    nc.vector.tensor_tensor(
                        ob[:, :], pj[:, :], bo_bc[:, nn * 512 : (nn + 1) * 512],
                        mybir.AluOpType.add,
                    )
                    if g < NQ - 1:
                        nc.sync.dma_start(
                            out=rs_in[
                                mt * 128 : (mt + 1) * 128, nn * 512 : (nn + 1) * 512
                            ],
                            in_=ob[:, :],
                        )
                    else:
                        nc.sync.dma_start(
                            out=out[
                                768 + (mt - 12) * 128 : 768 + (mt - 11) * 128,
                                nn * 512 : (nn + 1) * 512,
                            ],
                            in_=ob[:, :],
                        )
            if g < NQ - 1:
                nc.gpsimd.collective_compute(
                    "ReduceScatter",
                    mybir.AluOpType.add,
                    replica_groups=[[0, 1], [2, 3], [4, 5], [6, 7]],
                    ins=[rs_in[g0 : g0 + 512, :].opt()],
                    outs=[rs_out[g0 // 2 : g0 // 2 + 256, :].opt()],
                )
                nc.sync.dma_start(
                    out=out[g0 // 2 : g0 // 2 + 256, :],
                    in_=rs_out[g0 // 2 : g0 // 2 + 256, :],
                )

    nc.compile()
    return nc


def _get_nc():
    global _nc_cache
    if _nc_cache is None:
        _nc_cache = _build()
    return _nc_cache


def kernel(x, W_qkv, b_qkv, W_o, b_o):
    import ml_dtypes
    from concourse.bass_utils import run_bass_kernel_spmd

    bf = ml_dtypes.bfloat16
    x = np.asarray(x, dtype=np.float32)
    W_qkv = np.asarray(W_qkv, dtype=np.float32)
    b_qkv = np.asarray(b_qkv, dtype=np.float32)
    W_o = np.asarray(W_o, dtype=np.float32)
    b_o = np.asarray(b_o, dtype=np.float32)

    in_maps = []
    for c in range(N_CORES):
        b, g = divmod(c, 2)
        cs = slice(CL * g, CL * (g + 1))
        W_q_c = W_qkv[:, 0:C][:, cs]
        W_k_c = W_qkv[:, C : 2 * C][:, cs]
        W_v_c = W_qkv[:, 2 * C : 3 * C][:, cs]
        b_v_c = b_qkv[2 * C : 3 * C][cs]
        W_o_c = W_o[cs, :]
        # V-bias folds into the output bias: softmax rows sum to 1, so
        # P @ (1 b_v^T) = 1 b_v^T, and (O + 1 b_v^T) W_o = O W_o + 1 (b_v^T W_o).
        bo2 = 0.5 * b_o + b_v_c @ W_o_c
        in_maps.append(
            {
                "xt": np.ascontiguousarray(x[b].T).reshape(KC, 128, N).astype(bf),
                "w_q": np.ascontiguousarray(
                    W_q_c.reshape(KC, 128, PAIRS, 128).transpose(2, 1, 0, 3)
                ).astype(bf),
                "w_k": np.ascontiguousarray(
                    W_k_c.reshape(KC, 128, PAIRS, 128).transpose(2, 1, 0, 3)
                ).astype(bf),
                "w_v": np.ascontiguousarray(W_v_c.reshape(KC, 128, CL)).astype(bf),
                "w_o": np.ascontiguousarray(W_o_c.reshape(PAIRS, 128, C)).astype(bf),
                "b_q": np.ascontiguousarray(
                    b_qkv[0:C][cs].reshape(PAIRS, 128).T
                ).astype(np.float32),
                "b_o2": np.ascontiguousarray(bo2[None, :]).astype(np.float32),
                "tri": np.triu(np.ones((128, 128))).astype(bf),
            }
        )

    nc = _get_nc()
    trace = bool(int(os.environ.get("BASS_KERNEL_TRACE", "0")))
    tmpdir = os.environ.get("BASS_KERNEL_TRACE_DIR") or None
    res = run_bass_kernel_spmd(
        nc, in_maps, list(range(N_CORES)), trace=trace, tmpdir=tmpdir
    )
    kernel.last_result = res

    full = np.empty((B, N, C), dtype=np.float32)
    chunks = [(0, 512), (512, 512), (1024, 512)]
    outs = [np.asarray(res.results[c]["out"], dtype=np.float32) for c in range(N_CORES)]
    for c in range(N_CORES):
        b, rank = divmod(c, 2)
        o = outs[c]
        out_r = 0
        for t0, tn in chunks:
            h = tn // 2
            full[b, t0 + rank * h : t0 + (rank + 1) * h, :] = o[out_r : out_r + h, :]
            out_r += h
    for b in range(B):
        full[b, 1536:2048, :] = outs[2 * b][768:1280, :] + outs[2 * b + 1][768:1280, :]
    return full


kernel.last_result = None
